# revision 58
# baseline (speedup 1.0000x reference)
"""Multi-head attention on 8 TRN2 NeuronCores (Bass/Tile).

Problem: B=4, S=2048, D=1024, H=16 heads (DH=64).
  out = softmax((q@wq+bq)(k@wk+bk)^T / sqrt(H)) @ (v@wv+bv) @ wo + bo

Sharding: 8 cores = 4 batches x 2 head-groups (8 heads each). Each core
computes its batch's QKV projections restricted to its head group's
columns, attention for those 8 heads, and a partial output projection
(wo rows for its heads); the host sums partials per batch. Activations
are kept TRANSPOSED ([feature, seq]) on device so every matmul has its
contraction on the partition dim with no on-device transposes; the host
transposes inputs/outputs (cheap numpy).

v2 (this file): the whole kernel is a single software-pipelined stream.
All inputs and weights are bf16 (host-side cast; halves the DMA and
the input tensors stay resident in SBUF - no reloads). Only head-pair
0's projections run before the attention stream starts; everything
else - the remaining qh/kh projection chains, per-(key-tile,
head-pair) vh chains, and the wo partial chains - is emitted by
background generators popped into the attention loop under a
per-step cycle budget with step-count gates.

Scheduling rules found to matter (timeline-sim + hardware verified):
- The score-PSUM ring (2x [128,1024]) is reserved for the foreground
  QK matmuls only; EVERY background chain runs through one dedicated
  PSUM bank (bgps). Any stealing of the sc ring couples the next QK
  to the previous exp's completion (+~0.2-0.4us/step).
- ACT runs the exp stream exclusively (256 x [128,1024], ~272us);
  all wo-partial drains go to DVE. Putting drains on ACT paces the
  whole stream; Pool cannot read PSUM at all (BIR verifier).
- wo is split into 4 single-kt partials gated at steps 64/128/192/240
  so background PE work exists across the whole stream; partials are
  summed on the host. Each stream emits its qw0 chunks before qw1
  chunks: at gate time the gating head's qw1 finalize is not yet
  EMITTED (3-step PV pipeline lag) and an early-emitted qw1 read
  would miss dependency tracking and read uninitialized ctxT.
- Head order is 0,1,2,3,4,5,7,6: the last window's normalize then
  targets partitions 0-63 (direct DVE write) instead of the slower
  staging + gpsimd-DMA path for the upper half.
- The tail (kt3 x sc{2,3}, depends on the final window) rotates its
  PSUM through both the sc ring and the ctx ring with copies
  alternating ACT/DVE, and DMAs out per-m [128,1024] rows.

Per-step structure is the hardware-validated baseline: QK (2 matmuls,
K=64), exp (one ACT instr, PSUM->SBUF bf16), PV pairs with a 3-step
software-pipeline skew (depth 4 races on silicon!), normalize via
DVE reciprocal -> Pool partition_broadcast -> DVE multiply.

Engine budget (timeline sim): PE 334.6us busy (the binding resource),
ACT 271.9, DVE ~150, Pool ~45; total 391.4us vs 421.6us for the
phase-separated v1. Hardware-verified: HW exec 391432ns, rel err
8.56e-3 vs the fp32 reference (gate 2e-2).

Host: out[b] = (sum of bf16 partials outa+outc+outb[:, :S]+outb[:, S:]
of both cores).T + (bv @ wo + bo) (bv/bo fold exactly through the
linear tail since softmax rows sum to 1).

KNOWN UNEXPLOITED WIN (~45-55us, needs a full attention-core rewrite):
the PV matmul here is out=[65 part, 512 free] - only 65 of 128 output
partitions used, 50% PE waste, and cost is charged per free column.
Flipping it to out=[q-block 128 part, 65 free] with lhsT=et_block
[128 keys, 128 q] (stationary) and rhs=vh [128 keys, 65] (moving)
costs 65 cycles per block instead of 512: PV drops 262k -> 133k
cycles. The context then lands as [q, dh] so the softmax denominator
becomes a per-partition scalar (plain tensor_scalar_mul; no Pool
partition_broadcast, no upper-half staging DMA), and a [128,128]
XBAR DMA-transpose per head-PAIR q-block (free dim must be %128;
14ns/32x16 tile, runs on idle DMA engines) bridges back to the
[dl, seq] layout the wo chains need. Consequences that make it a
rewrite: q-windows must shrink to 512 so PSUM fits (sc 3x[128,512]
+ ctx 3x[128,260] + bg 2x[128,512] = 8 banks, and the bg bank
finally double-buffers); ACT grows to 512 exps x 601ns = 308us and
becomes the binding engine (total ~340-350us est.); head pairs must
interleave windows (0,1,0,1,...) so the pair-transpose staging ring
stays ~2KB/partition instead of 16KB; all stream gates rescale x2.
"""
import ml_dtypes
import numpy as np

import concourse.bass as bass
import concourse.mybir as mybir
from concourse import bacc
from concourse.tile import TileContext
from concourse.bass_utils import run_bass_kernel_spmd

B, S, D, H = 4, 2048, 1024, 16
DH = D // H          # 64
HG = H // 2          # 8 heads per core
DL = HG * DH         # 512 local qkv width
KT = D // 128        # 8 contraction tiles for projections
ST = S // 128        # 16 key tiles
QW = S // 1024       # 2 q windows of 1024
SCALE = 1.0 / np.sqrt(np.float32(H))  # 0.25

f32 = mybir.dt.float32
bf16 = mybir.dt.bfloat16
bf16_np = ml_dtypes.bfloat16


def _build_program() -> bacc.Bacc:
    nc = bacc.Bacc()
    qT_e = nc.declare_dram_parameter("qT", [D, S], bf16, isOutput=False)
    kT_e = nc.declare_dram_parameter("kT", [D, S], bf16, isOutput=False)
    vT_e = nc.declare_dram_parameter("vT", [D, S], bf16, isOutput=False)
    wq_e = nc.declare_dram_parameter("wq", [D, DL], bf16, isOutput=False)
    wk_e = nc.declare_dram_parameter("wk", [D, DL], bf16, isOutput=False)
    wv_e = nc.declare_dram_parameter("wv", [D, DL], bf16, isOutput=False)
    wo_e = nc.declare_dram_parameter("wo", [DL, D], bf16, isOutput=False)
    bq_e = nc.declare_dram_parameter("bq", [DL], f32, isOutput=False)
    bk_e = nc.declare_dram_parameter("bk", [DL], f32, isOutput=False)
    outa_e = nc.declare_dram_parameter("outTa", [D, S], bf16, isOutput=True)
    # outb holds TWO kt partials side by side (cols 0:S = kt2, S:2S = kt3)
    outb_e = nc.declare_dram_parameter("outTb", [D, 2 * S], bf16, isOutput=True)
    outc_e = nc.declare_dram_parameter("outTc", [D, S], bf16, isOutput=True)

    with TileContext(nc) as tc:
        with (
            tc.tile_pool(name="wp", bufs=1) as wpool,
            tc.tile_pool(name="xp", bufs=1) as xpool,
            tc.tile_pool(name="proj", bufs=1) as projpool,
            tc.tile_pool(name="attn", bufs=1) as attnpool,
            tc.tile_pool(name="sm", bufs=3) as smpool,
            tc.tile_pool(name="ps", bufs=2, space="PSUM") as pspool,
            tc.tile_pool(name="ctxps", bufs=3, space="PSUM") as ctxpspool,
            tc.tile_pool(name="bgps", bufs=1, space="PSUM") as bgpspool,
        ):
            # ---- weights / biases (persistent) ----
            # qkv weights stored as 4 tiles [128, 1024]: t-pair 2j|2j+1 side
            # by side (wider DMAs; SEQ-issue cost is per instruction)
            wt = {}
            for nm, ext in [("wv", wv_e), ("wq", wq_e), ("wk", wk_e)]:
                wt[nm] = [
                    wpool.tile([128, 2 * DL], bf16, tag=f"w{nm}", bufs=4,
                               name=f"{nm}t{t}")
                    for t in range(4)
                ]

            def wslice(nm, t, c0, w):
                # [128, w] view of contraction-tile t's columns c0:c0+w
                return wt[nm][t // 2][:, (t % 2) * DL + c0:
                                      (t % 2) * DL + c0 + w]
            wo_t = [
                wpool.tile([128, D], bf16, tag="wo", bufs=4, name=f"wo{t}")
                for t in range(4)
            ]
            bias_t = {}
            for nm in ("bq", "bk"):
                bias_t[nm] = wpool.tile([128, DL // 128], f32, tag=nm,
                                        name=f"b_{nm}")

            # resident inputs: per tensor 16 tiles [128, 1024] (t, half)
            xq = [xpool.tile([128, 1024], bf16, tag="xq", bufs=16,
                             name=f"xq{t}h{h}") for t in range(KT)
                  for h in range(2)]
            xk = [xpool.tile([128, 1024], bf16, tag="xk", bufs=16,
                             name=f"xk{t}h{h}") for t in range(KT)
                  for h in range(2)]
            xv = [xpool.tile([128, 1024], bf16, tag="xv", bufs=16,
                             name=f"xv{t}h{h}") for t in range(KT)
                  for h in range(2)]

            def xt(tiles, t, sc):
                # [128, 512] view of x contraction-tile t, seq chunk sc
                return tiles[t * 2 + sc // 2][:, (sc % 2) * 512:
                                              (sc % 2) * 512 + 512]

            def xt128(tiles, t, st):
                # [128, 128] view of x contraction-tile t, seq 128-tile st
                h, r = divmod(st, 8)
                return tiles[t * 2 + h][:, r * 128:r * 128 + 128]

            # projections: qh/kh per (m, sc) [128, 512] bf16
            qh = [projpool.tile([128, 512], bf16, tag="qh", bufs=16,
                                name=f"qh{m}s{sc}")
                  for m in range(4) for sc in range(4)]
            kh = [projpool.tile([128, 512], bf16, tag="kh", bufs=16,
                                name=f"kh{m}s{sc}")
                  for m in range(4) for sc in range(4)]
            # vh: per key-tile [128, 8 heads, 64+1(ones)] bf16
            vh_tiles = [
                projpool.tile([128, HG, 65], bf16, tag="vh", bufs=ST,
                              name=f"vh{st}")
                for st in range(ST)
            ]
            # ctxT: heads stacked on partitions, 4 tiles of [128, S] bf16
            ctxT = [
                attnpool.tile([128, S], bf16, tag="ctxT", bufs=4,
                              name=f"ctxT{t}")
                for t in range(4)
            ]

            # ---- DMA emission (single SP queue, consumption order) ----
            def dma_w(nm, ext):
                for j in range(4):
                    nc.sync.dma_start(
                        out=wt[nm][j].rearrange("p (a c) -> p a c", a=2),
                        in_=ext[2 * j * 128:(2 * j + 2) * 128, :].rearrange(
                            "(a p) c -> p a c", a=2))

            def dma_x(tiles, ext, half):
                for t in range(KT):
                    nc.sync.dma_start(
                        out=tiles[t * 2 + half],
                        in_=ext[t * 128:(t + 1) * 128,
                                half * 1024:(half + 1) * 1024])

            # consumption order: qh/kh m0 first, then v for the PV stream,
            # then the second halves in the order attention touches them.
            dma_w("wq", wq_e)
            dma_x(xq, qT_e, 0)
            dma_w("wk", wk_e)
            dma_x(xk, kT_e, 0)
            for nm, ext in [("bq", bq_e), ("bk", bk_e)]:
                nc.sync.dma_start(
                    out=bias_t[nm], in_=ext.rearrange("(j p) -> p j", p=128))
            dma_x(xk, kT_e, 1)
            dma_w("wv", wv_e)
            dma_x(xv, vT_e, 0)
            dma_x(xv, vT_e, 1)
            dma_x(xq, qT_e, 1)
            for t in range(4):
                nc.sync.dma_start(out=wo_t[t],
                                  in_=wo_e[t * 128:(t + 1) * 128, :])

            # ---- chain emitters ----
            def bg_ps(name):
                return bgpspool.tile([128, 512], f32, tag="bg", bufs=1,
                                     name=name)

            def proj_chain(kind, m, sc, fg=False):
                # qh/kh tile (m, sc) <- w[:, m-slice]^T @ x[:, sc-chunk]
                xtiles = xq if kind == "q" else xk
                dst = (qh if kind == "q" else kh)[m * 4 + sc]
                bias = bias_t["bq" if kind == "q" else "bk"]
                wnm = "wq" if kind == "q" else "wk"
                if fg:  # prefix: attention hasn't started, sc ring is free
                    ps = pspool.tile([128, 1024], f32, tag="sc", bufs=2,
                                     name=f"p{kind}{m}{sc}")
                else:
                    ps = bg_ps(f"p{kind}{m}{sc}")
                for t in range(KT):
                    nc.tensor.matmul(
                        ps[:, 0:512],
                        wslice(wnm, t, m * 128, 128),
                        xt(xtiles, t, sc),
                        start=(t == 0), stop=(t == KT - 1),
                    )
                nc.vector.tensor_scalar_add(
                    dst[:, :], ps[:, 0:512], bias[:, m:m + 1])

            def vh_chain(st, j, fg=False):
                # vh[st][:, 2j:2j+2, 0:64] <- x_v 128-slice @ wv head-pair j
                if fg:
                    ps = pspool.tile([128, 1024], f32, tag="sc", bufs=2,
                                     name=f"pv{st}j{j}")
                else:
                    ps = bg_ps(f"pv{st}j{j}")
                for t in range(KT):
                    nc.tensor.matmul(
                        ps[:, 0:128],
                        xt128(xv, t, st),
                        wslice("wv", t, j * 128, 128),
                        start=(t == 0), stop=(t == KT - 1),
                    )
                nc.vector.tensor_copy(
                    vh_tiles[st][:, 2 * j:2 * j + 2, 0:64],
                    ps.rearrange("p (u d) -> p u d", d=64)[:, 0:2, :],
                )
                if j == 0:
                    nc.gpsimd.memset(vh_tiles[st][:, :, 64:65], 1.0)

            def wo_chain(m, sc, kts, out_ext, tag, col0=0, force_act=None):
                # one output-projection chain over the given ctxT k-tiles
                ps = bg_ps(f"bg{tag}{m}{sc}")
                for i, t in enumerate(kts):
                    yield ("mm", 512, lambda t=t, i=i, ps=ps: nc.tensor.matmul(
                        ps[:, 0:512],
                        wo_t[t][:, m * 128:(m + 1) * 128],
                        ctxT[t][:, sc * 512:(sc + 1) * 512],
                        start=(i == 0), stop=(i == len(kts) - 1),
                    ))

                def drain(ps=ps, m=m, sc=sc):
                    # stage in the (long dead by now) xv input ring: saves
                    # 9KB/partition of SBUF vs dedicated staging tiles
                    ot = xpool.tile([128, 512], bf16, tag="xv", bufs=16,
                                    name=f"ot{tag}{m}{sc}")
                    eng = ("dve" if force_act is None else
                           ("act" if force_act else "dve"))
                    if eng == "act":
                        nc.scalar.copy(ot[:, :], ps[:, 0:512])
                    elif eng == "pool":
                        nc.gpsimd.tensor_copy(ot[:, :], ps[:, 0:512])
                    else:
                        nc.vector.tensor_copy(ot[:, :], ps[:, 0:512])
                    nc.sync.dma_start(
                        out=out_ext[m * 128:(m + 1) * 128,
                                    col0 + sc * 512:col0 + (sc + 1) * 512],
                        in_=ot[:, :],
                    )
                yield ("free", 0, drain)

            # ---- background stream generators ----
            def gen_proj(items):
                for kind, m, sc in items:
                    yield ("mm", 4096,
                           lambda kind=kind, m=m, sc=sc: proj_chain(kind, m, sc))

            def gen_vh(j, sts):
                for st in sts:
                    yield ("mm", 1024, lambda st=st, j=j: vh_chain(st, j))

            def gen_wo(kts, ext, tag, scs=(0, 1, 2, 3), ms=range(8), col0=0,
                       force_act=None):
                # qw0 chunks (sc 0,1) for ALL m first: at gate time the
                # gating head's qw1 finalize is not yet EMITTED (3-step PV
                # pipeline lag), so a qw1 read popped too early would skip
                # dependency tracking entirely and read uninitialized ctxT
                for group in ((0, 1), (2, 3)):
                    for m in ms:
                        for sc in group:
                            if sc in scs:
                                yield from wo_chain(m, sc, kts, ext, tag,
                                                    col0=col0,
                                                    force_act=force_act)

            # streams: [gate_step, generator, skip]. Projections and
            # vh have no attention dependencies - earliest gates, ordered by
            # deadline; wo partials gate on their ctxT k-tile's last head.
            def make_streams():
                return [
                    [0, gen_proj([("q", 0, 2), ("q", 0, 3)]), 0],
                    [0, gen_vh(1, range(ST)), 0],
                    [0, gen_proj([("k", 1, sc) for sc in range(4)]
                                 + [("q", 1, sc) for sc in range(4)]), 0],
                    [0, gen_vh(2, range(ST)), 0],
                    [0, gen_proj([("k", 2, sc) for sc in range(4)]
                                 + [("q", 2, sc) for sc in range(4)]), 0],
                    [0, gen_vh(3, range(ST)), 0],
                    [0, gen_proj([("k", 3, sc) for sc in range(4)]
                                 + [("q", 3, sc) for sc in range(4)]), 0],
                    [64, gen_wo([0], outa_e, "a"), 0],
                    [128, gen_wo([1], outc_e, "c"), 0],
                    [192, gen_wo([2], outb_e, "b0"), 0],
                    [240, gen_wo([3], outb_e, "b1", scs=(0, 1), col0=S,
                                 force_act=False), 3],
                ]

            def pop_bg(streams, step, budget):
                spent = 0
                for ent in streams:
                    if spent >= budget:
                        break
                    if step < ent[0] or ent[1] is None:
                        continue
                    if ent[2] > 0:
                        ent[2] -= 1
                        continue
                    while spent < budget:
                        try:
                            kind, cyc, go = next(ent[1])
                        except StopIteration:
                            ent[1] = None
                            break
                        go()
                        spent += cyc
                return spent

            # ---- attention stream ----
            def attention(streams):
                pending_pv = []  # (emit_fn, finalize_or_None)

                def flush_one_pv(depth=3):
                    if len(pending_pv) >= depth:
                        emit, fin = pending_pv.pop(0)
                        emit()
                        if fin is not None:
                            fin()

                def make_finalize(hl, qw, ctx_ps, ct_tile, hb):
                    def fin():
                        # normalize per 512 chunk: recip of sums row
                        # (psum@base64 -> sbuf@base0), broadcast, multiply
                        for c in range(2):
                            qoff = qw * 1024 + c * 512
                            rc = smpool.tile([1, 512], f32, tag="rc", bufs=1,
                                             name=f"rc{hl}{qw}{c}")
                            nc.vector.reciprocal(rc[0:1, :],
                                                 ctx_ps[c][64:65, :])
                            rb = smpool.tile([64, 512], f32, tag="rb", bufs=1,
                                             name=f"rb{hl}{qw}{c}")
                            nc.gpsimd.partition_broadcast(rb[:, :], rc[0:1, :])
                            if hb == 0:
                                nc.vector.tensor_mul(
                                    ct_tile[0:64, qoff:qoff + 512],
                                    ctx_ps[c][0:64, :], rb[:, :],
                                )
                            else:
                                stg = smpool.tile([64, 512], bf16, tag="stg",
                                                  bufs=2, name=f"stg{hl}{qw}{c}")
                                nc.vector.tensor_mul(stg[:, :],
                                                     ctx_ps[c][0:64, :],
                                                     rb[:, :])
                                nc.gpsimd.dma_start(
                                    out=ct_tile[hb:hb + 64, qoff:qoff + 512],
                                    in_=stg[:, :],
                                )
                    return fin

                step = [0]
                for hl in (0, 1, 2, 3, 4, 5, 7, 6):
                    qh_m = hl // 2
                    hb = (hl % 2) * 64
                    ct_tile = ctxT[hl // 2]
                    for qw in range(QW):
                        ctx_ps = [
                            ctxpspool.tile([65, 512], f32, tag="ctx", bufs=3,
                                           name=f"ctx{hl}{qw}{c}")
                            for c in range(2)
                        ]
                        for st in range(ST):
                            sc_ps = pspool.tile(
                                [128, 1024], f32, tag="sc", bufs=2,
                                name=f"sc{hl}{qw}{st}",
                            )
                            for half in range(2):
                                nc.tensor.matmul(
                                    sc_ps[:, half * 512:(half + 1) * 512],
                                    kh[qh_m * 4 + st // 4][
                                        hb:hb + 64,
                                        (st % 4) * 128:(st % 4) * 128 + 128],
                                    qh[qh_m * 4 + qw * 2 + half][hb:hb + 64, :],
                                    start=True, stop=True,
                                )
                            et = smpool.tile(
                                [128, 1024], bf16, tag="expT", bufs=4,
                                name=f"et{hl}{qw}{st}",
                            )
                            nc.scalar.activation(
                                et[:, :], sc_ps[:, :],
                                mybir.ActivationFunctionType.Exp,
                                scale=float(SCALE),
                            )
                            flush_one_pv()
                            # front-run the no-attention-dependency
                            # streams so PE never waits just-in-time
                            proj_live = any(streams[i][1] is not None
                                            for i in range(7))
                            budget = 4096 if proj_live else 1024
                            pop_bg(streams, step[0], budget)
                            step[0] += 1

                            def make_pv(st=st, et=et, ctx_ps=ctx_ps,
                                        vt=vh_tiles[st], hl=hl):
                                def emit():
                                    for half in range(2):
                                        nc.tensor.matmul(
                                            ctx_ps[half][:, :],
                                            vt[:, hl, :],
                                            et[:, half * 512:(half + 1) * 512],
                                            start=(st == 0), stop=(st == ST - 1),
                                        )
                                return emit
                            fin = (make_finalize(hl, qw, ctx_ps, ct_tile, hb)
                                   if st == ST - 1 else None)
                            pending_pv.append((make_pv(), fin))
                while pending_pv:
                    flush_one_pv(depth=1)

            # ---- schedule ----
            # prefix: head-pair 0's projections + vh, emitted in dependency
            # order; everything else is background inside attention.
            for sc in range(2):
                proj_chain("q", 0, sc, fg=True)
            for sc in range(4):
                proj_chain("k", 0, sc, fg=True)
            for st in range(ST):
                vh_chain(st, 0, fg=True)
            streams = make_streams()
            attention(streams)
            # leftover background (if any) + the outb sc 2,3 tail
            for ent in streams:
                if ent[1] is not None:
                    for kind, cyc, go in ent[1]:
                        go()
            tail_i = [0]

            def tail_ps(name):
                tail_i[0] += 1
                if tail_i[0] % 2:
                    return pspool.tile([128, 512], f32, tag="sc", bufs=2,
                                       name=name)
                return ctxpspool.tile([128, 512], f32, tag="ctx", bufs=3,
                                      name=name)

            for m in range(8):
                otm = xpool.tile([128, 1024], bf16, tag="xv", bufs=16,
                                 name=f"otail{m}")
                for sc in (2, 3):
                    ps = tail_ps(f"tail{m}{sc}")
                    nc.tensor.matmul(
                        ps[:, 0:512],
                        wo_t[3][:, m * 128:(m + 1) * 128],
                        ctxT[3][:, sc * 512:(sc + 1) * 512],
                        start=True, stop=True,
                    )
                    dst = otm[:, (sc - 2) * 512:(sc - 1) * 512]
                    if sc == 2:
                        nc.scalar.copy(dst, ps[:, 0:512])
                    else:
                        nc.vector.tensor_copy(dst, ps[:, 0:512])
                nc.sync.dma_start(
                    out=outb_e[m * 128:(m + 1) * 128, S + 1024:S + 2048],
                    in_=otm[:, :],
                )

    nc.compile()
    return nc


_NC = None


def _get_program():
    global _NC
    if _NC is None:
        _NC = _build_program()
    return _NC


def make_in_maps(q, k, v, wq, wk, wv, wo, bq, bk):
    in_maps = []
    for b in range(B):
        qT = np.ascontiguousarray(q[b].T).astype(bf16_np)
        kT = np.ascontiguousarray(k[b].T).astype(bf16_np)
        vT = np.ascontiguousarray(v[b].T).astype(bf16_np)
        for g in range(2):
            cols = slice(g * DL, (g + 1) * DL)
            in_maps.append({
                "qT": qT, "kT": kT, "vT": vT,
                "wq": np.ascontiguousarray(wq[:, cols]).astype(bf16_np),
                "wk": np.ascontiguousarray(wk[:, cols]).astype(bf16_np),
                "wv": np.ascontiguousarray(wv[:, cols]).astype(bf16_np),
                "wo": np.ascontiguousarray(wo[cols, :]).astype(bf16_np),
                "bq": np.ascontiguousarray(bq[cols]),
                "bk": np.ascontiguousarray(bk[cols]),
            })
    return in_maps


def assemble_out(results, wo, bv, bo):
    tail = bv @ wo + bo  # exact fold of v/output biases (softmax rows sum to 1)
    out = np.empty((B, S, D), np.float32)
    for b in range(B):
        acc = None
        for g in range(2):
            r = results[2 * b + g]
            part = (r["outTa"].astype(np.float32)
                    + r["outTc"].astype(np.float32)
                    + r["outTb"][:, :S].astype(np.float32)
                    + r["outTb"][:, S:].astype(np.float32))
            acc = part if acc is None else acc + part
        out[b] = acc.T + tail
    return out


def kernel(q, k, v, wq, bq, wk, bk, wv, bv, wo, bo, **_unused):
    q = np.asarray(q, np.float32)
    k = np.asarray(k, np.float32)
    v = np.asarray(v, np.float32)
    wq = np.asarray(wq, np.float32)
    wk = np.asarray(wk, np.float32)
    wv = np.asarray(wv, np.float32)
    wo = np.asarray(wo, np.float32)
    bq = np.asarray(bq, np.float32)
    bk = np.asarray(bk, np.float32)
    bv = np.asarray(bv, np.float32)
    bo = np.asarray(bo, np.float32)

    nc = _get_program()
    in_maps = make_in_maps(q, k, v, wq, wk, wv, wo, bq, bk)
    res = run_bass_kernel_spmd(nc, in_maps, core_ids=list(range(8))).results
    return assemble_out(res, wo, bv, bo)


if __name__ == "__main__":
    rng = np.random.default_rng(0)
    sd = 1.0 / np.sqrt(D)
    inputs = {
        "q": rng.standard_normal((B, S, D), dtype=np.float32),
        "k": rng.standard_normal((B, S, D), dtype=np.float32),
        "v": rng.standard_normal((B, S, D), dtype=np.float32),
        "wq": rng.standard_normal((D, D), dtype=np.float32) * sd,
        "bq": np.zeros(D, np.float32),
        "wk": rng.standard_normal((D, D), dtype=np.float32) * sd,
        "bk": np.zeros(D, np.float32),
        "wv": rng.standard_normal((D, D), dtype=np.float32) * sd,
        "bv": np.zeros(D, np.float32),
        "wo": rng.standard_normal((D, D), dtype=np.float32) * sd,
        "bo": np.zeros(D, np.float32),
    }
    out = kernel(**inputs)
    print("kernel ran:", out.shape, out.dtype)
